# revision 8
# baseline (speedup 1.0000x reference)
"""Trainium2 Bass kernel for a DoReFa-quantized ResNet BasicBlock (inference).

Reference computation (all fp32):
    out = qact(bn2(conv3x3(qact(bn1(conv3x3(x, qw(w1)))), qw(w2))) + x)
with qw = 4-bit DoReFa weight quant, qact = 4-bit activation quant,
x: (64, 128, 56, 56), convs 128->128 stride 1 pad 1.

Sharding: data-parallel over the batch dim, 8 images per NeuronCore on 8 cores.

Per-core kernel design (v2 — edge-time optimized over the 171us baseline):
  * HW truth: every matmul streams 1 PSUM elem/cycle at 2.4 GHz (PSUM-drain
    port bound; perf modes don't change it).  DoubleRow's win is doubled
    contraction (2 taps/pass).  PE floor = (9 conv1 + 5 conv2 passes) x
    448 x 7 x 8 images ~ 146 us; the kernel keeps that stream gap-free.
  * conv1 fp16 from a host-padded 15*x plane (fp8 taps measured rel-err
    0.09-0.12 — fails the 2e-2 gate; fp16 keeps it at 0.0106).
  * conv2 fp8e4m3 exact ints: 1 center + 3 DoubleRow dy-pairs + 1 DR
    dx-pair read from a shifted act1 duplicate.
  * Pair-major emission (v2): chunk pairs are emitted tap-major so every
    2nd matmul reuses the loaded weights — measured 189.2 ns vs ~192 ns
    per 448-elem matmul when weights change (NX dispatch + weight-load
    exposure).  PSUM: p1=4 + p2=4 banks.  Image 0's conv1 and the last
    image's conv2 stay per-chunk (input-DMA pacing / drain rotation).
  * Head (v2): DMA completion semaphores drain through a ~2us-each
    pipeline (~2 in flight), so the head uses FEW, large transfers on
    both HWDGE rings: 3 xf row bands on SP, w1/bn/w2 whole on ACT.  The
    44 warm-up matmuls bridge gap-free into the first real matmul
    (~12.2us, DMA-completion-bound) so the HAM throttle window is warm
    from the first real matmul.
  * Tail (v2): last image's conv2 chunks rotated [4,5,6,0,1,2,3]; final
    chunk's affine+piece halved; the last output-piece descriptors are
    split between the SP and ACT rings so descriptor generation (~0.6us
    each) is not serialized behind one queue.
  * bn1 folds to Relu(s*psum+b) on ScalarE writing fp16 v1; DVE min15
    then two +2^23 round-ops write the two fp8 act1 copies; bn2 + fp16
    residual add + round + clamp per piece, fp8 ints out, host /15.
  * Fixed overheads measured: ~7.3us NEFF preamble before the kernel body
    branches, ~2us DMA completion latency per transfer, ~8.1us exit
    epilogue (EVENT_SEMAPHORE sweep, invariant).

Measured baseline (v1): 171-173us HW exec, rel L2 err 0.0106.
"""

import os
import sys

import numpy as np

for _p in ("/opt/trn_rl_repo", "/opt/pypackages"):
    if _p not in sys.path and os.path.isdir(_p):
        sys.path.insert(0, _p)

import ml_dtypes  # noqa: E402

# ---------------------------------------------------------------- constants
B, C, H, W = 64, 128, 56, 56
N_CORES = 8
BPC = B // N_CORES          # images per core
RPC = 8                     # output rows per PSUM chunk
NCHUNK = H // RPC           # 7 chunks
FREE = RPC * W              # 448 PSUM elems per chunk
XP = H + 2                  # padded x row length (58)
XPLANE = XP * XP            # 3364 fp16 elems per padded x plane
AW = 64                     # act1 padded row pitch (bytes, fp8)
AIMG = (H + 2) * AW         # 3712 padded act1 plane bytes
AB = 1                      # act1 base offset: keeps round-op dst offsets even
ACT_D = 3726                # shifted act1 copy offset; pair step +2 % 16 == 0
ATW = ACT_D + AB + AIMG     # act1 tile width (orig + shifted copy)
PLANE = H * W               # 3136
MAGIC = float(2**23)        # fp32 round-to-nearest-even magic constant
EPS = 1e-5

# image-0 xf row bands (pad-row units).  DMA completion semaphores fire
# through a ~2us-per-completion pipeline, so FEW large bands beat many
# small ones; band boundaries chosen so each lands just before the chunk
# that needs it.
X0_ROWS = (0, 22, 46, 58)

_CACHE = {}


# ---------------------------------------------------------------- host math
def _quant_weight_int(w):
    """Return 15*quantize_weight(w, 4): exact odd integers in [-15, 15]."""
    wt = np.tanh(w.astype(np.float64)).astype(np.float32)
    m = np.float32(np.abs(wt).max())
    wtn = wt / (np.float32(2.0) * m) + np.float32(0.5)
    q = np.round(wtn * np.float32(15.0)).astype(np.float32)
    return np.float32(2.0) * q - np.float32(15.0)


def _bn_affine(gamma, beta, mean, var):
    inv = 1.0 / np.sqrt(var.astype(np.float64) + EPS)
    s = gamma.astype(np.float64) * inv
    b = beta.astype(np.float64) - mean.astype(np.float64) * s
    return s, b


def _lhsT_taps(w_int):
    """[oc, ic, 3, 3] -> [ic, 9*oc] stationary layout, tap-major."""
    t = np.transpose(w_int, (2, 3, 1, 0)).reshape(9, C, C)   # [tap, ic, oc]
    return np.transpose(t, (1, 0, 2)).reshape(C, 9 * C)


# conv1 tap emission order; w1p is laid out in THIS order (center first:
# full-coverage start=True; the (1,0) stop tap last).
EMIT_ORDER = [(0, 0), (-1, -1), (-1, 0), (-1, 1), (0, -1),
              (0, 1), (1, -1), (1, 1), (1, 0)]
POS1 = {t: i for i, t in enumerate(EMIT_ORDER)}


def _chunk_seq(cch):
    """conv1 tap order for one chunk: center first (start, full coverage),
    a full-coverage dy-tap last (stop)."""
    last = (1, 0) if cch < NCHUNK - 1 else (-1, 0)
    seq = [(0, 0)] + [t for t in EMIT_ORDER[1:] if t != last]
    seq.append(last)
    return seq


# ---------------------------------------------------------------- bass build
def _split_multiwaits(nc, mybir):
    """Walrus encodes at most ONE sync wait per instruction; hoist extras
    onto same-engine NoOps placed immediately before."""
    nid = 0
    for fn in nc.m.functions:
        for blk in fn.blocks:
            out = []
            changed = False
            for ins in blk.instructions:
                si = ins.sync_info
                if si is not None and len(si.on_wait) > 1:
                    waits = list(si.on_wait)
                    for w in waits[:-1]:
                        nid += 1
                        nop = mybir.InstNoOp(name=f"I-wfix-{nid}",
                                             engine=ins.engine)
                        nop.sync_info = mybir.SyncInfo(on_wait=[w],
                                                       on_update=[])
                        out.append(nop)
                    ins.sync_info = mybir.SyncInfo(
                        on_wait=[waits[-1]], on_update=list(si.on_update))
                    changed = True
                out.append(ins)
            if changed:
                blk.instructions = out


def _build_module(apply_wfix=True):
    import concourse.bass as bass
    import concourse.mybir as mybir
    import concourse.tile as tile
    from contextlib import ExitStack

    f32 = mybir.dt.float32
    f16 = mybir.dt.float16
    f8 = mybir.dt.float8e4
    AF = mybir.ActivationFunctionType
    OP = mybir.AluOpType
    DR = mybir.MatmulPerfMode.DoubleRow

    nc = bass.Bass("TRN2", target_bir_lowering=False, debug=False,
                   num_devices=N_CORES)

    xf_d = nc.dram_tensor("xf", [BPC, C, XPLANE], f16, kind="ExternalInput")
    w1_d = nc.dram_tensor("w1p", [C, 9 * C], f16, kind="ExternalInput")
    w2_d = nc.dram_tensor("w2p", [C, 9 * C], f8, kind="ExternalInput")
    bn_d = nc.dram_tensor("bnv", [C, 4], f32, kind="ExternalInput")
    out_d = nc.dram_tensor("out", [BPC, C, PLANE], f8, kind="ExternalOutput")

    with tile.TileContext(nc) as tc, ExitStack() as ctx:
        const = ctx.enter_context(tc.tile_pool(name="const", bufs=1))
        sb = ctx.enter_context(tc.tile_pool(name="sb", bufs=3))
        ps = ctx.enter_context(tc.tile_pool(name="ps", bufs=4, space="PSUM"))

        # const loads on the ACT HWDGE ring (parallel with SP's xf bands);
        # one DMA each — completion semaphores are the scarce resource
        w1_sb = const.tile([C, 9 * C], f16)
        nc.scalar.dma_start(w1_sb[:], w1_d.ap())
        bn_sb = const.tile([C, 4], f32)
        nc.scalar.dma_start(bn_sb[:], bn_d.ap())
        w2_sb = const.tile([C, 9 * C], f8)
        nc.scalar.dma_start(w2_sb[:], w2_d.ap())
        sc1, bi1 = bn_sb[:, 0:1], bn_sb[:, 1:2]
        sc2, bi2 = bn_sb[:, 2:3], bn_sb[:, 3:4]

        # PE p-state warm-up: the tensor engine needs ~3.4us of continuous
        # work to un-throttle to 2.4GHz; run short dummy matmuls on a small
        # zeroed tile during the input-DMA wait
        warm = const.tile([C, 256], f8)
        nc.gpsimd.memset(warm[:], 0.0)
        wps = ps.tile([C, 112], f32, tag="p1", name="warm_ps", bufs=4)
        NWARM = 44   # bridge to ~12.2us so no PE-idle gap before the
        #              first real matmul re-arms the HAM throttle window
        for k in range(NWARM):
            mv = bass.AP(tensor=warm.tensor, offset=0,
                         ap=[[256, C], [1, 112]])
            nc.tensor.matmul(wps[:], lhsT=warm[:, 128:256], rhs=mv,
                             start=(k == 0), stop=(k == NWARM - 1))

        # two static act1 slots; pad borders zeroed once
        act1a = const.tile([C, ATW], f8)
        act1b = const.tile([C, ATW], f8)
        for a1 in (act1a, act1b):
            for base in (AB, ACT_D + AB):
                r = a1[:, base:base + AIMG].rearrange("p (h w) -> p h w", w=AW)
                nc.gpsimd.memset(r[:, 0:1, :], 0.0)        # pad row 0
                nc.gpsimd.memset(r[:, 57:58, :], 0.0)      # pad row 57
                nc.gpsimd.memset(r[:, 1:57, 0], 0.0)       # pad col 0
                nc.gpsimd.memset(r[:, 1:57, 57:64], 0.0)   # dead cols

        def mm1(xf, p1, cch, dy, dx, start, stop):
            """One conv1 tap matmul, trimmed to nonzero products."""
            r0 = RPC * cch
            t9 = POS1[(dy, dx)]
            rlo = max(r0, -dy)
            rhi = min(r0 + RPC - 1, H - 1 - dy)
            nr = rhi - rlo + 1
            j0 = 0 if dx >= 0 else 1
            nj = W - abs(dx)
            off = (rlo + dy + 1) * XP + 1 + j0 + dx
            mv = bass.AP(tensor=xf.tensor, offset=off,
                         ap=[[XPLANE, C], [XP, nr], [1, nj]])
            out = bass.AP(tensor=p1.tensor,
                          offset=(rlo - r0) * W + j0,
                          ap=[[FREE, C], [W, nr], [1, nj]])
            nc.tensor.matmul(out, lhsT=w1_sb[:, t9 * C:(t9 + 1) * C],
                             rhs=mv, start=start, stop=stop)

        def bn1(v1, p1, cch):
            # bn1: Relu(s*psum+b) -> v1 fp16 (lower clip for free)
            nc.scalar.activation(v1[:, FREE * cch:FREE * (cch + 1)],
                                 p1[:], AF.Relu, bias=bi1, scale=sc1)

        def emit_load_conv1(n):
            """Load image n, conv1 + bn1 + qact; returns (v1, xf, act1-slot)."""
            xf = sb.tile([C, XPLANE], f16, tag="xf", name=f"xf_{n}")
            if n == 0:
                # row bands on SP so chunk 0's matmuls start early
                for rb0, rb1 in zip(X0_ROWS[:-1], X0_ROWS[1:]):
                    nc.sync.dma_start(xf[:, rb0 * XP:rb1 * XP],
                                      xf_d.ap()[n][:, rb0 * XP:rb1 * XP])
            else:
                nc.sync.dma_start(xf[:], xf_d.ap()[n])

            v1 = sb.tile([C, PLANE], f16, tag="v1", name=f"v1_{n}")
            a1 = act1a if n % 2 == 0 else act1b

            if n == 0:
                # per-chunk emission: chunk c's matmuls pace the band DMAs
                for cch in range(NCHUNK):
                    if cch == 5:
                        emit_round_half(n, v1, a1, 0)
                    p1 = ps.tile([C, FREE], f32, tag="p1",
                                 name=f"p1_{n}_{cch}", bufs=4)
                    for i, (dy, dx) in enumerate(_chunk_seq(cch)):
                        mm1(xf, p1, cch, dy, dx, i == 0, i == 8)
                    bn1(v1, p1, cch)
                return v1, xf, a1

            # pair-major: both chunks of a pair emitted tap-major so every
            # 2nd matmul reuses the loaded weights
            for pi, (ca, cb) in enumerate(((0, 1), (2, 3), (4, 5))):
                if pi == 2:
                    emit_round_half(n, v1, a1, 0)
                pa = ps.tile([C, FREE], f32, tag="p1",
                             name=f"p1_{n}_{ca}", bufs=4)
                pb = ps.tile([C, FREE], f32, tag="p1",
                             name=f"p1_{n}_{cb}", bufs=4)
                seq = _chunk_seq(ca)          # ca, cb < 6: same sequence
                for i, (dy, dx) in enumerate(seq):
                    mm1(xf, pa, ca, dy, dx, i == 0, i == 8)
                    mm1(xf, pb, cb, dy, dx, i == 0, i == 8)
                bn1(v1, pa, ca)
                bn1(v1, pb, cb)
            p1 = ps.tile([C, FREE], f32, tag="p1", name=f"p1_{n}_6", bufs=4)
            for i, (dy, dx) in enumerate(_chunk_seq(6)):
                mm1(xf, p1, 6, dy, dx, i == 0, i == 8)
            bn1(v1, p1, 6)
            return v1, xf, a1

        def emit_round_half(n, v1, a1, h):
            # upper clip + round-to-int into both fp8 act1 copies, by half:
            # conv2's early chunks unblock as soon as half 0 lands
            r0, r1 = (0, 28) if h == 0 else (28, 56)
            sl = slice(W * r0, W * r1)
            nc.vector.tensor_scalar_min(v1[:, sl], v1[:, sl], 15.0)
            v1r = v1[:].rearrange("p (h w) -> p h w", w=W)
            for base in (AB, ACT_D + AB):
                ar = a1[:, base:base + AIMG].rearrange("p (h w) -> p h w",
                                                       w=AW)
                nc.vector.tensor_scalar(ar[:, r0 + 1:r1 + 1, 1:57],
                                        v1r[:, r0:r1, :], MAGIC, MAGIC,
                                        op0=OP.add, op1=OP.subtract)

        def mm2(a1, p2, cch, op, start, stop):
            """One conv2 op: op 0 = center single, 1..3 = DR dy-pairs
            (dx = -1,0,1), 4 = DR dx-pair via the shifted act1 copy."""
            r0 = RPC * cch
            if op == 0:
                mv = bass.AP(tensor=a1.tensor,
                             offset=AB + (r0 + 1) * AW + 1,
                             ap=[[ATW, C], [AW, RPC], [1, W]])
                nc.tensor.matmul(p2[:], lhsT=w2_sb[:, 8 * C:9 * C], rhs=mv,
                                 start=start, stop=stop)
            elif op <= 3:
                dxi = op - 1
                dx = (-1, 0, 1)[dxi]
                j0 = 0 if dx >= 0 else 1
                nj = W - abs(dx)
                mv = bass.AP(tensor=a1.tensor,
                             offset=AB + r0 * AW + 1 + j0 + dx,
                             ap=[[ATW, C], [2 * AW, 2], [AW, RPC], [1, nj]])
                wpair = w2_sb[:, dxi * 2 * C:(dxi + 1) * 2 * C].rearrange(
                    "p (two m) -> p two m", two=2)
                out = bass.AP(tensor=p2.tensor, offset=j0,
                              ap=[[FREE, C], [W, RPC], [1, nj]])
                nc.tensor.matmul(out, lhsT=wpair, rhs=mv, perf_mode=DR,
                                 start=start, stop=stop)
            else:
                mv = bass.AP(tensor=a1.tensor, offset=AB + (r0 + 1) * AW,
                             ap=[[ATW, C], [ACT_D + 2, 2], [AW, RPC],
                                 [1, W]])
                wpair = w2_sb[:, 6 * C:8 * C].rearrange(
                    "p (two m) -> p two m", two=2)
                nc.tensor.matmul(p2[:], lhsT=wpair, rhs=mv, perf_mode=DR,
                                 start=start, stop=stop)

        def emit_conv2_out(n, xf, a1):
            """conv2 + bn2 + residual + qact for image n, DMA fp8 ints out."""
            v2 = sb.tile([C, PLANE], f16, tag="v2", name=f"v2_{n}")
            ost = sb.tile([C, PLANE], f8, tag="ost", name=f"ost_{n}")
            od = out_d.ap()[n]
            xfr = xf[:].rearrange("p (h w) -> p h w", w=XP)

            def emit_piece(r0, r1, eng=nc.sync):
                sl = slice(W * r0, W * r1)
                nc.vector.tensor_tensor(v2[:, sl], v2[:, sl],
                                        xfr[:, r0 + 1:r1 + 1, 1:57],
                                        op=OP.add)
                nc.vector.tensor_scalar(v2[:, sl], v2[:, sl], MAGIC, MAGIC,
                                        op0=OP.add, op1=OP.subtract)
                nc.vector.tensor_scalar(ost[:, sl], v2[:, sl], 0.0, 15.0,
                                        op0=OP.max, op1=OP.min)
                eng.dma_start(od[:, sl], ost[:, sl])

            def bn2(cch):
                nc.scalar.activation(v2[:, FREE * cch:FREE * (cch + 1)],
                                     p2t[cch][:], AF.Identity,
                                     bias=bi2, scale=sc2)

            p2t = {}
            if n < BPC - 1:
                # pair-major: consecutive matmuls share weights
                piece_after = {0: (0, 14), 1: (14, 28), 2: (28, 48)}
                for pi, (ca, cb) in enumerate(((0, 1), (2, 3), (4, 5))):
                    for cch in (ca, cb):
                        p2t[cch] = ps.tile([C, FREE], f32, tag="p2",
                                           name=f"p2_{n}_{cch}", bufs=4)
                    for op in range(5):
                        mm2(a1, p2t[ca], ca, op, op == 0, op == 4)
                        mm2(a1, p2t[cb], cb, op, op == 0, op == 4)
                    bn2(ca)
                    bn2(cb)
                    if pi in piece_after:
                        emit_piece(*piece_after[pi])
                p2t[6] = ps.tile([C, FREE], f32, tag="p2",
                                 name=f"p2_{n}_6", bufs=4)
                for op in range(5):
                    mm2(a1, p2t[6], 6, op, op == 0, op == 4)
                bn2(6)
                emit_piece(48, 56)
                return

            # last image: rotated per-chunk emission so every output piece
            # except the final 8 rows completes while the stream still runs
            order = [4, 5, 6, 0, 1, 2, 3]
            piece_after = {1: (32, 48), 2: (48, 56), 3: (0, 8),
                           4: (8, 16), 5: (16, 24)}
            for pos, cch in enumerate(order):
                p2 = ps.tile([C, FREE], f32, tag="p2", name=f"p2_{n}_{cch}",
                             bufs=4)
                p2t[cch] = p2
                for op in range(5):
                    mm2(a1, p2, cch, op, op == 0, op == 4)
                if pos == NCHUNK - 1:
                    # final chunk: halve the affine+piece; the second
                    # half's output descriptor goes on the ACT ring so the
                    # two final descriptors are generated in parallel
                    for hh in range(2):
                        sl = slice(FREE * cch + 224 * hh,
                                   FREE * cch + 224 * (hh + 1))
                        nc.scalar.activation(v2[:, sl], p2[:, 224 * hh:
                                                           224 * (hh + 1)],
                                             AF.Identity, bias=bi2, scale=sc2)
                        emit_piece(RPC * cch + 4 * hh, RPC * cch + 4 * (hh + 1),
                                   eng=(nc.sync if hh == 0 else nc.scalar))
                    continue
                nc.scalar.activation(v2[:, FREE * cch:FREE * (cch + 1)],
                                     p2[:], AF.Identity, bias=bi2, scale=sc2)
                if pos in piece_after:
                    emit_piece(*piece_after[pos])

        prev = None
        for s in range(BPC + 1):
            cur = emit_load_conv1(s) if s < BPC else None
            if cur is not None:
                emit_round_half(s, cur[0], cur[2], 1)
            if prev is not None:
                emit_conv2_out(s - 1, prev[1], prev[2])
            prev = cur

    if apply_wfix:
        _split_multiwaits(nc, mybir)
    return nc


def _get_module(apply_wfix=True):
    key = ("nc", apply_wfix)
    if key not in _CACHE:
        _CACHE[key] = _build_module(apply_wfix)
    return _CACHE[key]


# ---------------------------------------------------------------- host entry
def _make_in_maps(x, w1, w2, gamma1, beta1, mean1, var1,
                  gamma2, beta2, mean2, var2):
    F8 = ml_dtypes.float8_e4m3
    x15 = np.float32(15.0) * np.asarray(x, np.float32)
    x15 = x15.reshape(N_CORES, BPC, C, H, W)

    # padded fp16 plane: rows/cols 1..56 live, zero borders
    xf = np.zeros((N_CORES, BPC, C, XP, XP), np.float16)
    xf[..., 1:57, 1:57] = x15.astype(np.float16)
    xf = xf.reshape(N_CORES, BPC, C, XPLANE)

    w1i = _quant_weight_int(np.asarray(w1, np.float32))
    w2i = _quant_weight_int(np.asarray(w2, np.float32))
    w1t = _lhsT_taps(w1i)
    tap1 = lambda dy, dx: w1t[:, ((dy + 1) * 3 + dx + 1) * C:
                              ((dy + 1) * 3 + dx + 1 + 1) * C]
    w1p = np.concatenate([tap1(dy, dx) for dy, dx in EMIT_ORDER],
                         axis=1).astype(np.float16)
    w2t = _lhsT_taps(w2i)
    tap2 = lambda t9: w2t[:, t9 * C:(t9 + 1) * C]
    # conv2: 3 (dy=-1,dx)+(dy=+1,dx) pairs, the dy=0 dx=-1/+1 pair, center
    blocks = []
    for dxi in range(3):
        blocks += [tap2(dxi), tap2(6 + dxi)]
    blocks += [tap2(3), tap2(5), tap2(4)]
    w2p = np.concatenate(blocks, axis=1).astype(F8)

    s1, b1 = _bn_affine(np.asarray(gamma1, np.float32),
                        np.asarray(beta1, np.float32),
                        np.asarray(mean1, np.float32),
                        np.asarray(var1, np.float32))
    s2, b2 = _bn_affine(np.asarray(gamma2, np.float32),
                        np.asarray(beta2, np.float32),
                        np.asarray(mean2, np.float32),
                        np.asarray(var2, np.float32))
    # PSUM holds 225*conv (15x and 15w) -> affine to 15*bn
    bnv = np.stack([s1 / 15.0, 15.0 * b1, s2 / 15.0, 15.0 * b2],
                   axis=1).astype(np.float32)

    shared = {"w1p": w1p, "w2p": w2p, "bnv": bnv}
    return [{"xf": np.ascontiguousarray(xf[i]), **shared}
            for i in range(N_CORES)]


def kernel(**inputs):
    from concourse.bass_utils import run_bass_kernel_spmd

    nc = _get_module()
    in_maps = _make_in_maps(**inputs)
    res = run_bass_kernel_spmd(nc, in_maps, core_ids=list(range(N_CORES)))
    _CACHE["last_res"] = res
    # exact: out fp8 ints k in 0..15 -> f32 k/15
    out = np.concatenate(
        [np.asarray(r["out"]).astype(np.float32) / np.float32(15.0)
         for r in res.results], axis=0)
    return out.reshape(B, C, H, W)
